# revision 35
# baseline (speedup 1.0000x reference)
"""Trainium2 Bass kernel for nn_Attention_v4 (sparse per-atom attention).

Reference computation (fp32):
    x:[2,512,14,1024] -> qkv = x@w_qkv+b_qkv -> per (b, r=atom, head) attention
    over the n=512 axis -> out @ w_proj + b_proj.

Sharding (8 cores): 4 groups x 7 (b,r)-units data-parallel, x 2 head-halves
tensor-parallel. Each core computes, for its 7 units and its 8 heads:
QKV^T projection, attention, and a partial c_proj (contraction over its 512
of the 1024 hd rows). Host unshard sums the two head-half partials (the
"all-reduce" of the TP split) and adds b_proj.

Device layouts (matmuls in float32r: full PE rate at N>=256):
  qk   [col(q), tok]            - w-stationary projection, q pair c in tile c
                                  (even head rows 0-63, odd rows 64-127)
  kpack[qdim, pair, j]          - k^T packed per pair: even head on
                                  partitions 0-63, odd on 64-127; scores are
                                  K=64 row-tiled matmuls (measured: the two
                                  heads do NOT overlap in the array through
                                  this stack, but the packed layout still
                                  avoids the zero-pad init and half-copies)
  v    [tok, lh*65+d]           - 65th col per head = 1.0 -> P@V also yields
                                  softmax denominators as row 64
  S^T  [j, i] pairs             - one 2-bank psum tile per (pair, jt); ONE
                                  exp instruction covers both heads
  O^T  [hd, i]                  - po drained to sbuf fast (frees psum bank);
                                  denominator row replicated via a K=1 PE
                                  matmul, reciprocal as a 64-partition DVE op
                                  (a [1,512] DVE reciprocal is 3.2us on HW,
                                  gpsimd broadcast saturates Pool)
  out  [tok, e] partial         - c_proj of unit u-1 interleaved into the
                                  attention steps of unit u as PE filler;
                                  per step: PV+drains first, fillers, then
                                  new STs (keeps psum frees ahead of fresh
                                  deps in the strict-FIFO engine queues)
"""

import numpy as np

B, N, A, DIM, H, D = 2, 512, 14, 1024, 16, 64
HL = 8            # heads per core
UNITS = 7         # (b, r) units per group
NCORES = 8
SCALE = np.float32(1.0 / np.sqrt(np.sqrt(D)))
VW = D + 1        # v width per head incl. ones column

_CACHE = {}


def _build_nc(units=UNITS, repeat=1, phases="QAC", qk_bias=False):
    import concourse.bacc as bacc
    import concourse.tile as tile
    from concourse import mybir
    from concourse.bass import ts

    f32, f32r = mybir.dt.float32, mybir.dt.float32r
    AF = mybir.ActivationFunctionType

    nc = bacc.Bacc("TRN2", target_bir_lowering=False, debug=False,
                   num_devices=NCORES)
    xT = nc.dram_tensor("xT", [units, DIM, N], f32r, kind="ExternalInput")
    wqkv = nc.dram_tensor("wqkv", [DIM, 1024 + HL * D], f32r,
                          kind="ExternalInput")
    bqk = nc.dram_tensor("bqk", [1024], f32, kind="ExternalInput")
    bv = nc.dram_tensor("bv", [HL * VW], f32, kind="ExternalInput")
    wproj = nc.dram_tensor("wproj", [HL * D, DIM], f32r, kind="ExternalInput")
    part = nc.dram_tensor("part", [units, N, DIM], f32, kind="ExternalOutput")
    # scratch for the denominator-row broadcast bounce (sbuf->dram->sbuf;
    # sbuf->sbuf DMA cannot use a stride-0 partition AP)
    dscr = nc.dram_tensor("dscr", [4, 2, N], f32, kind="Internal")

    import concourse.bass as bass

    def bcast_part(ap, p=128):
        # replicate a 1D DRAM vector across p partitions (step-0 partition dim)
        return bass.AP(tensor=ap.tensor, offset=ap.offset,
                       ap=[[0, p]] + list(ap.ap))

    with tile.TileContext(nc) as tc:
        import contextlib
        with contextlib.ExitStack() as ctx:
            const = ctx.enter_context(tc.tile_pool(name="const", bufs=1))
            p_x = ctx.enter_context(tc.tile_pool(name="p_x", bufs=2))
            p_qk = ctx.enter_context(tc.tile_pool(name="p_qk", bufs=2))
            p_v = ctx.enter_context(tc.tile_pool(name="p_v", bufs=2))
            p_es = ctx.enter_context(tc.tile_pool(name="p_es", bufs=4))
            p_ot = ctx.enter_context(tc.tile_pool(name="p_ot", bufs=2))
            p_or = ctx.enter_context(tc.tile_pool(name="p_or", bufs=4))
            p_out = ctx.enter_context(tc.tile_pool(name="p_out", bufs=2))
            p_rc = ctx.enter_context(tc.tile_pool(name="p_rc", bufs=2))
            # 8 psum banks: 2 (proj + c_proj groups) + 4 (score pairs) + 2 (PV)
            ps_mm = ctx.enter_context(
                tc.tile_pool(name="ps_mm", bufs=2, space="PSUM"))
            ps_st = ctx.enter_context(
                tc.tile_pool(name="ps_st", bufs=2, space="PSUM"))
            ps_o = ctx.enter_context(
                tc.tile_pool(name="ps_o", bufs=2, space="PSUM"))

            # ---- persistent weights ----
            wq_sb = const.tile([128, 8, 1024 + HL * D], f32r, tag="wqkv")
            _wq_r = wqkv[:].rearrange("(k p) c -> p k c", p=128)
            for k in range(8):
                nc.sync.dma_start(out=wq_sb[:, k, :], in_=_wq_r[:, k, :])
            wp_sb = const.tile([128, 4, DIM], f32r, tag="wproj")
            nc.sync.dma_start(
                out=wp_sb, in_=wproj[:].rearrange("(k p) c -> p k c", p=128))
            bqk_sb = const.tile([128, 8], f32, tag="bqk")
            nc.sync.dma_start(
                out=bqk_sb, in_=bqk[:].rearrange("(c p) -> p c", p=128))
            bv_sb = const.tile([128, HL * VW], f32, tag="bv")
            nc.sync.dma_start(out=bv_sb, in_=bcast_part(bv[:]))
            # packed k^T: pair c at [:, c, :], even head on partitions 0-63,
            # odd head on 64-127 (same layout the q/k projection emits)
            if "Z" in phases:   # probe: zero-padded K=128 serial scores
                kpack = const.tile([128, 2, 4, N], f32r, tag="kpack")
                nc.vector.memset(kpack[64:128, 0, :, :].bitcast(f32), 0.0)
                nc.vector.memset(kpack[0:64, 1, :, :].bitcast(f32), 0.0)
            else:
                kpack = const.tile([128, 4, N], f32r, tag="kpack")
            # ones column for the K=1 broadcast matmul (replicates the
            # reciprocal denominator row across 64 partitions on the PE --
            # gpsimd partition_broadcast has a per-op overhead that
            # saturates Pool and cascades into PV stalls)
            ones_sb = const.tile([128, 64], f32r, tag="ones")
            nc.vector.memset(ones_sb[:].bitcast(f32), 1.0)

            def prefetch_x(u):
                x_sb = p_x.tile([128, 8, N], f32r, tag="x", name="x_sb")
                nc.sync.dma_start(
                    out=x_sb, in_=xT[u].rearrange("(k p) n -> p k n", p=128))
                return x_sb

            def proj_body(u, x_sb, after_first_group=None):
                """QKV projection for unit u (x_sb prefetched earlier)."""
                qk_sb = p_qk.tile([128, 4, N], f32r, tag="qk")
                for ct in range(8):
                    pm = ps_mm.tile([128, N], f32, tag="mm")
                    for k in range(8):
                        nc.tensor.matmul(
                            pm, wq_sb[:, k, ts(ct, 128)], x_sb[:, k, :],
                            start=(k == 0), stop=(k == 7))
                    if ct >= 4 and "Z" in phases:
                        c = ct - 4
                        nc.vector.tensor_copy(
                            out=kpack[0:64, 0, c, :], in_=pm[0:64, :])
                        nc.vector.tensor_copy(
                            out=kpack[64:128, 1, c, :], in_=pm[64:128, :])
                    else:
                        dst = (qk_sb[:, ct, :] if ct < 4
                               else kpack[:, ct - 4, :])
                        if qk_bias:
                            nc.vector.tensor_scalar_add(
                                dst, pm, bqk_sb[:, ct:ct + 1])
                        else:
                            nc.vector.tensor_copy(out=dst, in_=pm)
                    if ct == 0 and after_first_group is not None:
                        after_first_group()

                v_sb = p_v.tile([128, 4, HL * VW], f32r, tag="v")
                vv = v_sb.rearrange("p t (h w) -> p t h w", w=VW)
                bvv = bv_sb.rearrange("p (h w) -> p h w", w=VW)
                for tt in range(4):
                    pv = ps_mm.tile([128, N], f32, tag="mm")
                    pvv = pv.rearrange("p (h d) -> p h d", d=D)
                    for k in range(8):
                        nc.tensor.matmul(
                            pv, x_sb[:, k, ts(tt, 128)],
                            wq_sb[:, k, 1024:1024 + HL * D],
                            start=(k == 0), stop=(k == 7))
                    nc.vector.tensor_add(
                        out=vv[:, tt, :, 0:D], in0=pvv, in1=bvv[:, :, 0:D])
                # the ones-rider column (one 32-element op for all tt/h)
                nc.vector.tensor_scalar(
                    out=vv[:, :, :, D],
                    in0=bv_sb[:, 0:32].rearrange("p (a b) -> p a b", b=8),
                    scalar1=0.0, scalar2=1.0,
                    op0=mybir.AluOpType.mult, op1=mybir.AluOpType.add)
                return qk_sb, v_sb

            def cproj_gen(u, ot_sb):
                """c_proj of unit u: generator yielding after each matmul so
                it can be interleaved into the next unit's attention steps.
                ct-outer / eh-inner: consecutive matmuls share the stationary
                ot tile, halving LDWEIGHTS."""
                for tt in range(4):
                    o_sb = p_out.tile([128, DIM], f32, tag="out")
                    pc0 = ps_mm.tile([128, N], f32, tag="mm", name="pc0")
                    pc1 = ps_mm.tile([128, N], f32, tag="mm", name="pc1")
                    for ct in range(4):
                        for eh, pc in ((0, pc0), (1, pc1)):
                            nc.tensor.matmul(
                                pc, ot_sb[:, ct, ts(tt, 128)],
                                wp_sb[:, ct, eh * 512:(eh + 1) * 512],
                                start=(ct == 0), stop=(ct == 3))
                            yield
                    for eh, pc in ((0, pc0), (1, pc1)):
                        nc.vector.tensor_copy(
                            out=o_sb[:, eh * 512:(eh + 1) * 512], in_=pc)
                    nc.sync.dma_start(out=part[u, ts(tt, 128), :], in_=o_sb)

            def attn_body(u, qk_sb, v_sb, filler, fake_es=None):
                """Attention for unit u, with c_proj(u-1) matmuls interleaved
                as PE filler work (2 per pair-step)."""
                ot_sb = p_ot.tile([128, 4, N], f32r, tag="ot")
                steps = [(c, jt) for c in range(4) for jt in range(4)]
                ess = {}
                pos = {}
                pend_bc = []

                def emit_st(c, jt):
                    pst = ps_st.tile([128, 2, N], f32, tag="st")
                    if "Z" in phases:
                        for half in range(2):
                            nc.tensor.matmul(
                                pst[:, half, :],
                                kpack[:, half, c, ts(jt, 128)],
                                qk_sb[:, c, :], start=True, stop=True)
                    else:
                        nc.tensor.matmul(
                            pst[:, 0, :], kpack[0:64, c, ts(jt, 128)],
                            qk_sb[0:64, c, :], start=True, stop=True,
                            tile_position=(0, 0))
                        nc.tensor.matmul(
                            pst[:, 1, :], kpack[64:128, c, ts(jt, 128)],
                            qk_sb[64:128, c, :], start=True, stop=True,
                            tile_position=(64, 0))
                    if "F" in phases:       # probe: no exp at all
                        ess[(c, jt)] = fake_es
                        return
                    es_t = p_es.tile([128, 2, N], f32r, tag="es")
                    nc.scalar.activation(out=es_t, in_=pst, func=AF.Exp)
                    ess[(c, jt)] = fake_es if "G" in phases else es_t

                def emit_pav(c, jt):
                    if jt == 0:
                        po_e = ps_o.tile([128, N], f32, tag="o", name="po_e")
                        po_o = ps_o.tile([128, N], f32, tag="o", name="po_o")
                        pos[c] = (po_e, po_o)
                    es_t = ess.pop((c, jt))
                    for half in range(2):
                        lh = 2 * c + half
                        nc.tensor.matmul(
                            pos[c][half][0:VW, :],
                            v_sb[:, jt, lh * VW:(lh + 1) * VW],
                            es_t[:, half, :], start=(jt == 0), stop=(jt == 3))
                    if jt == 3:
                        po_e, po_o = pos.pop(c)
                        if "H" in phases:   # probe: skip normalize chain
                            nc.vector.tensor_copy(
                                out=ot_sb[:, c, :], in_=po_e)
                            return
                        # drain po to sbuf promptly (frees the psum banks);
                        # split across ACT/DVE to balance engine load
                        or_e = p_or.tile([VW, N], f32r, tag="or")
                        or_o = p_or.tile([VW, N], f32r, tag="or")
                        nc.scalar.activation(out=or_e, in_=po_e[0:VW, :],
                                             func=AF.Copy)
                        nc.vector.tensor_copy(out=or_o, in_=po_o[0:VW, :])
                        pend_bc.append((c, or_e, or_o))

                def flush_bc():
                    # Deferred ~2 steps so nothing waits on the po->sbuf
                    # copies. An sbuf->sbuf DMA with a stride-0 partition AP
                    # replicates the RAW denominator row across 64
                    # partitions (off every compute engine); the reciprocal
                    # then runs as a full-64-partition op (a [1,512] DVE
                    # reciprocal measures 3.2us on HW; gpsimd broadcast
                    # saturates Pool).
                    c, or_e, or_o = pend_bc.pop(0)
                    bc2 = p_rc.tile([64, 2, N], f32, tag="bc2")
                    for half, orr in ((0, or_e), (1, or_o)):
                        nc.sync.dma_start(out=dscr[c, half, :],
                                          in_=orr[64:65, :].bitcast(f32))
                    for half in range(2):
                        nc.sync.dma_start(
                            out=bc2[:, half, :],
                            in_=bcast_part(dscr[c, half, :], p=64))
                    rb = p_rc.tile([64, 2, N], f32, tag="rb")
                    for half, orr in ((0, or_e), (1, or_o)):
                        nc.vector.reciprocal_approx_fast(
                            out=rb[:, half, :], in_=bc2[:, half, :])
                        bp = half * 64
                        nc.vector.tensor_mul(
                            out=ot_sb[bp:bp + 64, c, :],
                            in0=orr[0:64, :], in1=rb[:, half, :])

                # per step: PV (+po drains) first -- their deps are oldest --
                # then filler c_proj matmuls, then the new STs whose psum
                # slot / exp are the freshest deps. This keeps the po->or
                # drain copies ahead of the new exp in the ACT FIFO so the
                # next pair's PV never waits on a psum bank free.
                LOOKAHEAD = 3
                for s in range(len(steps) + LOOKAHEAD):
                    if s >= LOOKAHEAD:
                        emit_pav(*steps[s - LOOKAHEAD])
                    if filler is not None:
                        next(filler, None)
                        next(filler, None)
                    if s < len(steps):
                        emit_st(*steps[s])
                    if pend_bc and s % 4 == 1:
                        flush_bc()
                if filler is not None:
                    for _ in filler:
                        pass

                def flush_rest():
                    while pend_bc:
                        flush_bc()
                return ot_sb, flush_rest

            fake_es = None
            if "F" in phases or "G" in phases:
                fake_es = const.tile([128, 2, N], f32r, tag="fakees")
                nc.vector.memset(fake_es[:].bitcast(f32), 0.001)

            def whole_body():
                filler = None
                flush_rest = None
                x_next = prefetch_x(0)
                for u in range(units):
                    x_cur = x_next
                    if u + 1 < units:
                        x_next = prefetch_x(u + 1)
                    qk_sb, v_sb = proj_body(
                        u, x_cur, after_first_group=flush_rest)
                    if "A" in phases:
                        ot_sb, flush_rest = attn_body(
                            u, qk_sb, v_sb,
                            filler if "C" in phases else None,
                            fake_es=fake_es)
                        filler = cproj_gen(u, ot_sb)
                    else:
                        # probe mode: dump raw qk as the "output"
                        for tt in range(4):
                            nc.sync.dma_start(
                                out=part[u, ts(tt, 128), :],
                                in_=qk_sb[:, 0:2, :].bitcast(f32))
                if flush_rest is not None:
                    flush_rest()
                if "A" in phases and "C" in phases and filler is not None:
                    for _ in filler:
                        pass

            if repeat == 1:
                whole_body()
            else:
                with tc.For_i(0, repeat, 1):
                    whole_body()

    nc.compile()
    return nc


def _make_runner(nc, n_cores=NCORES, donate=True):
    """Persistent jitted SPMD runner (mirrors bass2jax.run_bass_via_pjrt)."""
    import jax
    from jax.sharding import Mesh, PartitionSpec
    from jax.experimental.shard_map import shard_map
    from concourse import bass2jax
    from concourse import mybir as mb

    bass2jax.install_neuronx_cc_hook()
    pn = nc.partition_id_tensor.name if nc.partition_id_tensor else None
    in_names, out_names, out_avals, out_shapes = [], [], [], []
    for alloc in nc.m.functions[0].allocations:
        if not isinstance(alloc, mb.MemoryLocationSet):
            continue
        name = alloc.memorylocations[0].name
        if alloc.kind == "ExternalInput":
            if name != pn:
                in_names.append(name)
        elif alloc.kind == "ExternalOutput":
            shape = tuple(alloc.tensor_shape)
            dtype = mb.dt.np(alloc.dtype)
            out_names.append(name)
            out_avals.append(jax.core.ShapedArray(shape, dtype))
            out_shapes.append((shape, dtype))
    n_params = len(in_names)
    n_outs = len(out_names)
    all_in = list(in_names) + list(out_names) + ([pn] if pn else [])

    def _body(*args):
        ops = list(args)
        if pn:
            ops.append(bass2jax.partition_id_tensor())
        return tuple(bass2jax._bass_exec_p.bind(
            *ops, out_avals=tuple(out_avals), in_names=tuple(all_in),
            out_names=tuple(out_names), lowering_input_output_aliases=(),
            sim_require_finite=True, sim_require_nnan=True, nc=nc))

    devices = jax.devices()[:n_cores]
    mesh = Mesh(np.asarray(devices), ("core",))
    specs = (PartitionSpec("core"),)
    fn = jax.jit(
        shard_map(_body, mesh=mesh, in_specs=specs * (n_params + n_outs),
                  out_specs=specs * n_outs, check_rep=False),
        donate_argnums=tuple(range(n_params, n_params + n_outs)) if donate else (),
        keep_unused=True)

    def run(in_maps):
        per_core = [[np.asarray(m[name]) for name in in_names] for m in in_maps]
        concat_in = [np.concatenate([per_core[c][i] for c in range(n_cores)],
                                    axis=0) for i in range(n_params)]
        concat_zeros = [np.zeros((n_cores * s[0], *s[1:]), d)
                        for (s, d) in out_shapes]
        import jax as _jax
        out_arrs = _jax.block_until_ready(fn(*concat_in, *concat_zeros))
        return [
            {name: np.asarray(out_arrs[i]).reshape(n_cores, *out_shapes[i][0])[c]
             for i, name in enumerate(out_names)}
            for c in range(n_cores)
        ]

    run.jit_fn = fn
    run.in_names = in_names
    run.out_names = out_names
    run.out_shapes = out_shapes
    run.n_cores = n_cores
    return run


def _unit_groups():
    units = [(b, r) for b in range(B) for r in range(A)]
    return [units[g * UNITS:(g + 1) * UNITS] for g in range(4)]


def shard_inputs(x, w_qkv, b_qkv, w_proj, b_proj):
    groups = _unit_groups()
    w4 = w_qkv.reshape(DIM, H, 3, D)
    b4 = b_qkv.reshape(H, 3, D)
    in_maps = []
    for c in range(NCORES):
        g, hh = c // 2, c % 2
        heads = list(range(hh * HL, (hh + 1) * HL))
        xT = np.ascontiguousarray(
            np.stack([x[b, :, r, :].T for (b, r) in groups[g]])
        ).astype(np.float32)
        wq = w4[:, heads, 0, :].reshape(DIM, HL * D) * SCALE
        wk = w4[:, heads, 1, :].reshape(DIM, HL * D) * SCALE
        wv = w4[:, heads, 2, :].reshape(DIM, HL * D)
        wqkv_c = np.ascontiguousarray(
            np.concatenate([wq, wk, wv], axis=1)).astype(np.float32)
        bq = (b4[heads, 0, :].reshape(HL * D) * SCALE)
        bk = (b4[heads, 1, :].reshape(HL * D) * SCALE)
        bvv = np.concatenate([b4[heads, 2, :], np.ones((HL, 1), np.float32)],
                             axis=1).reshape(HL * VW)
        in_maps.append({
            "xT": xT,
            "wqkv": wqkv_c,
            "bqk": np.concatenate([bq, bk]).astype(np.float32),
            "bv": bvv.astype(np.float32),
            "wproj": np.ascontiguousarray(
                w_proj[hh * HL * D:(hh + 1) * HL * D, :]).astype(np.float32),
        })
    return in_maps


def unshard(results, b_proj):
    groups = _unit_groups()
    out = np.zeros((B, N, A, DIM), np.float32)
    for g in range(4):
        s = results[2 * g]["part"] + results[2 * g + 1]["part"]
        for idx, (b, r) in enumerate(groups[g]):
            out[b, :, r, :] = s[idx]
    return out + b_proj.astype(np.float32)


def get_runner(qk_bias=False):
    key = ("runner", qk_bias)
    if key not in _CACHE:
        nc = _build_nc(qk_bias=qk_bias)
        _CACHE[key] = _make_runner(nc)
    return _CACHE[key]


def kernel(x, w_qkv, b_qkv, w_proj, b_proj):
    x = np.asarray(x)
    w_qkv = np.asarray(w_qkv)
    b_qkv = np.asarray(b_qkv)
    w_proj = np.asarray(w_proj)
    b_proj = np.asarray(b_proj)
    run = get_runner(qk_bias=bool(np.any(b_qkv[:2048])))
    in_maps = shard_inputs(x, w_qkv, b_qkv, w_proj, b_proj)
    results = run(in_maps)
    return unshard(results, b_proj)


# revision 39
# speedup vs baseline: 1.0424x; 1.0424x over previous
"""Trainium2 Bass kernel for nn_Attention_v4 (sparse per-atom attention).

Reference computation (fp32):
    x:[2,512,14,1024] -> qkv = x@w_qkv+b_qkv -> per (b, r=atom, head) attention
    over the n=512 axis -> out @ w_proj + b_proj.

Sharding (8 cores): 4 groups x 7 (b,r)-units data-parallel, x 2 head-halves
tensor-parallel. Each core computes, for its 7 units and its 8 heads:
QKV^T projection, attention, and a partial c_proj (contraction over its 512
of the 1024 hd rows). Host unshard sums the two head-half partials (the
"all-reduce" of the TP split) and adds b_proj.

Device layouts (matmuls in float32r: full PE rate at N>=256):
  qk   [col(q), tok]            - w-stationary projection, q pair c in tile c
                                  (even head rows 0-63, odd rows 64-127)
  kpack[qdim, pair, j]          - k^T packed per pair: even head on
                                  partitions 0-63, odd on 64-127; scores are
                                  K=64 row-tiled matmuls (measured: the two
                                  heads do NOT overlap in the array through
                                  this stack, but the packed layout still
                                  avoids the zero-pad init and half-copies)
  v    [tok, lh*65+d]           - 65th col per head = 1.0 -> P@V also yields
                                  softmax denominators as row 64
  S^T  [j, i] pairs             - one 2-bank psum tile per (pair, jt); ONE
                                  exp instruction covers both heads
  O^T  [hd, i]                  - po drained to sbuf fast (frees psum bank);
                                  denominator row replicated via a K=1 PE
                                  matmul, reciprocal as a 64-partition DVE op
                                  (a [1,512] DVE reciprocal is 3.2us on HW,
                                  gpsimd broadcast saturates Pool)
  out  [tok, e] partial         - c_proj of unit u-1 interleaved into the
                                  attention steps of unit u as PE filler;
                                  per step: PV+drains first, fillers, then
                                  new STs (keeps psum frees ahead of fresh
                                  deps in the strict-FIFO engine queues)
"""

import numpy as np

B, N, A, DIM, H, D = 2, 512, 14, 1024, 16, 64
HL = 8            # heads per core
UNITS = 7         # (b, r) units per group
NCORES = 8
SCALE = np.float32(1.0 / np.sqrt(np.sqrt(D)))
VW = D + 1        # v width per head incl. ones column

_CACHE = {}


def _build_nc(units=UNITS, repeat=1, phases="QAC", qk_bias=False):
    import concourse.bacc as bacc
    import concourse.tile as tile
    from concourse import mybir
    from concourse.bass import ts

    f32, f32r = mybir.dt.float32, mybir.dt.float32r
    AF = mybir.ActivationFunctionType

    nc = bacc.Bacc("TRN2", target_bir_lowering=False, debug=False,
                   num_devices=NCORES)
    xT = nc.dram_tensor("xT", [units, DIM, N], f32r, kind="ExternalInput")
    wqkv = nc.dram_tensor("wqkv", [DIM, 1024 + HL * D], f32r,
                          kind="ExternalInput")
    bqk = nc.dram_tensor("bqk", [1024], f32, kind="ExternalInput")
    bv = nc.dram_tensor("bv", [HL * VW], f32, kind="ExternalInput")
    wproj = nc.dram_tensor("wproj", [HL * D, DIM], f32r, kind="ExternalInput")
    part = nc.dram_tensor("part", [units, N, DIM], f32, kind="ExternalOutput")

    import concourse.bass as bass

    def bcast_part(ap, p=128):
        # replicate a 1D DRAM vector across p partitions (step-0 partition dim)
        return bass.AP(tensor=ap.tensor, offset=ap.offset,
                       ap=[[0, p]] + list(ap.ap))

    with tile.TileContext(nc) as tc:
        import contextlib
        with contextlib.ExitStack() as ctx:
            const = ctx.enter_context(tc.tile_pool(name="const", bufs=1))
            p_x = ctx.enter_context(tc.tile_pool(name="p_x", bufs=2))
            p_qk = ctx.enter_context(tc.tile_pool(name="p_qk", bufs=2))
            p_v = ctx.enter_context(tc.tile_pool(name="p_v", bufs=2))
            p_es = ctx.enter_context(tc.tile_pool(name="p_es", bufs=4))
            p_ot = ctx.enter_context(tc.tile_pool(name="p_ot", bufs=2))
            p_or = ctx.enter_context(tc.tile_pool(name="p_or", bufs=4))
            p_out = ctx.enter_context(tc.tile_pool(name="p_out", bufs=2))
            p_rc = ctx.enter_context(tc.tile_pool(name="p_rc", bufs=2))
            # 8 psum banks: 2 (proj + c_proj groups) + 4 (score pairs) + 2 (PV)
            ps_mm = ctx.enter_context(
                tc.tile_pool(name="ps_mm", bufs=2, space="PSUM"))
            ps_st = ctx.enter_context(
                tc.tile_pool(name="ps_st", bufs=2, space="PSUM"))
            ps_o = ctx.enter_context(
                tc.tile_pool(name="ps_o", bufs=2, space="PSUM"))

            # ---- persistent weights ----
            wq_sb = const.tile([128, 8, 1024 + HL * D], f32r, tag="wqkv")
            _wq_r = wqkv[:].rearrange("(k p) c -> p k c", p=128)
            for k in range(8):
                nc.sync.dma_start(out=wq_sb[:, k, :], in_=_wq_r[:, k, :])
            wp_sb = const.tile([128, 4, DIM], f32r, tag="wproj")
            nc.sync.dma_start(
                out=wp_sb, in_=wproj[:].rearrange("(k p) c -> p k c", p=128))
            bqk_sb = const.tile([128, 8], f32, tag="bqk")
            nc.sync.dma_start(
                out=bqk_sb, in_=bqk[:].rearrange("(c p) -> p c", p=128))
            bv_sb = const.tile([128, HL * VW], f32, tag="bv")
            nc.sync.dma_start(out=bv_sb, in_=bcast_part(bv[:]))
            # packed k^T: pair c at [:, c, :], even head on partitions 0-63,
            # odd head on 64-127 (same layout the q/k projection emits)
            if "Z" in phases:   # probe: zero-padded K=128 serial scores
                kpack = const.tile([128, 2, 4, N], f32r, tag="kpack")
                nc.vector.memset(kpack[64:128, 0, :, :].bitcast(f32), 0.0)
                nc.vector.memset(kpack[0:64, 1, :, :].bitcast(f32), 0.0)
            else:
                kpack = const.tile([128, 4, N], f32r, tag="kpack")
            # ones column for the K=1 broadcast matmul (replicates the
            # reciprocal denominator row across 64 partitions on the PE --
            # gpsimd partition_broadcast has a per-op overhead that
            # saturates Pool and cascades into PV stalls)
            ones_sb = const.tile([128, 64], f32r, tag="ones")
            nc.vector.memset(ones_sb[:].bitcast(f32), 1.0)

            def prefetch_x(u):
                x_sb = p_x.tile([128, 8, N], f32r, tag="x", name="x_sb")
                nc.sync.dma_start(
                    out=x_sb, in_=xT[u].rearrange("(k p) n -> p k n", p=128))
                return x_sb

            def proj_body(u, x_sb, after_first_group=None):
                """QKV projection for unit u (x_sb prefetched earlier)."""
                qk_sb = p_qk.tile([128, 4, N], f32r, tag="qk")
                for ct in range(8):
                    pm = ps_mm.tile([128, N], f32, tag="mm")
                    for k in range(8):
                        nc.tensor.matmul(
                            pm, wq_sb[:, k, ts(ct, 128)], x_sb[:, k, :],
                            start=(k == 0), stop=(k == 7))
                    if ct >= 4 and "Z" in phases:
                        c = ct - 4
                        nc.vector.tensor_copy(
                            out=kpack[0:64, 0, c, :], in_=pm[0:64, :])
                        nc.vector.tensor_copy(
                            out=kpack[64:128, 1, c, :], in_=pm[64:128, :])
                    else:
                        dst = (qk_sb[:, ct, :] if ct < 4
                               else kpack[:, ct - 4, :])
                        if qk_bias:
                            nc.vector.tensor_scalar_add(
                                dst, pm, bqk_sb[:, ct:ct + 1])
                        else:
                            nc.vector.tensor_copy(out=dst, in_=pm)
                    if ct == 0 and after_first_group is not None:
                        after_first_group()

                v_sb = p_v.tile([128, 4, HL * VW], f32r, tag="v")
                vv = v_sb.rearrange("p t (h w) -> p t h w", w=VW)
                bvv = bv_sb.rearrange("p (h w) -> p h w", w=VW)
                for tt in range(4):
                    pv = ps_mm.tile([128, N], f32, tag="mm")
                    pvv = pv.rearrange("p (h d) -> p h d", d=D)
                    for k in range(8):
                        nc.tensor.matmul(
                            pv, x_sb[:, k, ts(tt, 128)],
                            wq_sb[:, k, 1024:1024 + HL * D],
                            start=(k == 0), stop=(k == 7))
                    nc.vector.tensor_add(
                        out=vv[:, tt, :, 0:D], in0=pvv, in1=bvv[:, :, 0:D])
                # the ones-rider column (one 32-element op for all tt/h)
                nc.vector.tensor_scalar(
                    out=vv[:, :, :, D],
                    in0=bv_sb[:, 0:32].rearrange("p (a b) -> p a b", b=8),
                    scalar1=0.0, scalar2=1.0,
                    op0=mybir.AluOpType.mult, op1=mybir.AluOpType.add)
                return qk_sb, v_sb

            def cproj_gen(u, ot_sb):
                """c_proj of unit u: generator yielding after each matmul so
                it can be interleaved into the next unit's attention steps.
                ct-outer / eh-inner: consecutive matmuls share the stationary
                ot tile, halving LDWEIGHTS."""
                for tt in range(4):
                    o_sb = p_out.tile([128, DIM], f32, tag="out")
                    pc0 = ps_mm.tile([128, N], f32, tag="mm", name="pc0")
                    pc1 = ps_mm.tile([128, N], f32, tag="mm", name="pc1")
                    for ct in range(4):
                        for eh, pc in ((0, pc0), (1, pc1)):
                            nc.tensor.matmul(
                                pc, ot_sb[:, ct, ts(tt, 128)],
                                wp_sb[:, ct, eh * 512:(eh + 1) * 512],
                                start=(ct == 0), stop=(ct == 3))
                            yield
                    for eh, pc in ((0, pc0), (1, pc1)):
                        nc.vector.tensor_copy(
                            out=o_sb[:, eh * 512:(eh + 1) * 512], in_=pc)
                    nc.sync.dma_start(out=part[u, ts(tt, 128), :], in_=o_sb)

            def attn_body(u, qk_sb, v_sb, filler, fake_es=None):
                """Attention for unit u, with c_proj(u-1) matmuls interleaved
                as PE filler work (2 per pair-step)."""
                ot_sb = p_ot.tile([128, 4, N], f32r, tag="ot")
                steps = [(c, jt) for c in range(4) for jt in range(4)]
                ess = {}
                pos = {}
                pend_bc = []

                def emit_st(c, jt):
                    pst = ps_st.tile([128, 2, N], f32, tag="st")
                    if "Z" in phases:
                        for half in range(2):
                            nc.tensor.matmul(
                                pst[:, half, :],
                                kpack[:, half, c, ts(jt, 128)],
                                qk_sb[:, c, :], start=True, stop=True)
                    else:
                        nc.tensor.matmul(
                            pst[:, 0, :], kpack[0:64, c, ts(jt, 128)],
                            qk_sb[0:64, c, :], start=True, stop=True,
                            tile_position=(0, 0))
                        nc.tensor.matmul(
                            pst[:, 1, :], kpack[64:128, c, ts(jt, 128)],
                            qk_sb[64:128, c, :], start=True, stop=True,
                            tile_position=(64, 0))
                    if "F" in phases:       # probe: no exp at all
                        ess[(c, jt)] = fake_es
                        return
                    es_t = p_es.tile([128, 2, N], f32r, tag="es")
                    nc.scalar.activation(out=es_t, in_=pst, func=AF.Exp)
                    ess[(c, jt)] = fake_es if "G" in phases else es_t

                def emit_pav(c, jt):
                    if jt == 0:
                        po_e = ps_o.tile([128, N], f32, tag="o", name="po_e")
                        po_o = ps_o.tile([128, N], f32, tag="o", name="po_o")
                        pos[c] = (po_e, po_o)
                    es_t = ess.pop((c, jt))
                    for half in range(2):
                        lh = 2 * c + half
                        nc.tensor.matmul(
                            pos[c][half][0:VW, :],
                            v_sb[:, jt, lh * VW:(lh + 1) * VW],
                            es_t[:, half, :], start=(jt == 0), stop=(jt == 3))
                    if jt == 3:
                        po_e, po_o = pos.pop(c)
                        if "H" in phases:   # probe: skip normalize chain
                            nc.vector.tensor_copy(
                                out=ot_sb[:, c, :], in_=po_e)
                            return
                        # drain po to sbuf promptly (frees the psum banks);
                        # split across ACT/DVE to balance engine load
                        or_e = p_or.tile([VW, N], f32r, tag="or")
                        or_o = p_or.tile([VW, N], f32r, tag="or")
                        nc.scalar.activation(out=or_e, in_=po_e[0:VW, :],
                                             func=AF.Copy)
                        nc.vector.tensor_copy(out=or_o, in_=po_o[0:VW, :])
                        pend_bc.append((c, or_e, or_o))

                def flush_bc():
                    # Deferred ~2 steps so the in-order PE queue never waits
                    # on the po->sbuf copies. The K=1 matmul replicates the
                    # RAW denominator row across 64 partitions; the
                    # reciprocal then runs as a full-64-partition op (a
                    # [1,512] DVE reciprocal measures 3.2us on HW; gpsimd
                    # broadcast saturates Pool; a DRAM-bounce DMA broadcast
                    # measured slower than the K=1 matmul).
                    c, or_e, or_o = pend_bc.pop(0)
                    bc2 = ps_st.tile([128, 2, N], f32, tag="st", name="bc2")
                    for half, orr in ((0, or_e), (1, or_o)):
                        nc.tensor.matmul(
                            bc2[0:64, half, :], ones_sb[64:65, :],
                            orr[64:65, :], start=True, stop=True)
                    rb = p_rc.tile([64, 2, N], f32, tag="rb")
                    for half, orr in ((0, or_e), (1, or_o)):
                        nc.vector.reciprocal_approx_fast(
                            out=rb[:, half, :], in_=bc2[0:64, half, :])
                        bp = half * 64
                        nc.vector.tensor_mul(
                            out=ot_sb[bp:bp + 64, c, :],
                            in0=orr[0:64, :], in1=rb[:, half, :])

                # per step: PV (+po drains) first -- their deps are oldest --
                # then filler c_proj matmuls, then the new STs whose psum
                # slot / exp are the freshest deps. This keeps the po->or
                # drain copies ahead of the new exp in the ACT FIFO so the
                # next pair's PV never waits on a psum bank free.
                LOOKAHEAD = 3
                for s in range(len(steps) + LOOKAHEAD):
                    if s >= LOOKAHEAD:
                        emit_pav(*steps[s - LOOKAHEAD])
                    if filler is not None:
                        next(filler, None)
                        next(filler, None)
                    if s < len(steps):
                        emit_st(*steps[s])
                    if pend_bc and s % 4 == 1:
                        flush_bc()
                if filler is not None:
                    for _ in filler:
                        pass

                def flush_rest():
                    while pend_bc:
                        flush_bc()
                return ot_sb, flush_rest

            fake_es = None
            if "F" in phases or "G" in phases:
                fake_es = const.tile([128, 2, N], f32r, tag="fakees")
                nc.vector.memset(fake_es[:].bitcast(f32), 0.001)

            def whole_body():
                filler = None
                flush_rest = None
                x_next = prefetch_x(0)
                for u in range(units):
                    x_cur = x_next
                    if u + 1 < units:
                        x_next = prefetch_x(u + 1)
                    qk_sb, v_sb = proj_body(
                        u, x_cur, after_first_group=flush_rest)
                    if "A" in phases:
                        ot_sb, flush_rest = attn_body(
                            u, qk_sb, v_sb,
                            filler if "C" in phases else None,
                            fake_es=fake_es)
                        filler = cproj_gen(u, ot_sb)
                    else:
                        # probe mode: dump raw qk as the "output"
                        for tt in range(4):
                            nc.sync.dma_start(
                                out=part[u, ts(tt, 128), :],
                                in_=qk_sb[:, 0:2, :].bitcast(f32))
                if flush_rest is not None:
                    flush_rest()
                if "A" in phases and "C" in phases and filler is not None:
                    for _ in filler:
                        pass

            if repeat == 1:
                whole_body()
            else:
                with tc.For_i(0, repeat, 1):
                    whole_body()

    nc.compile()
    return nc


def _make_runner(nc, n_cores=NCORES, donate=True):
    """Persistent jitted SPMD runner (mirrors bass2jax.run_bass_via_pjrt)."""
    import jax
    from jax.sharding import Mesh, PartitionSpec
    from jax.experimental.shard_map import shard_map
    from concourse import bass2jax
    from concourse import mybir as mb

    bass2jax.install_neuronx_cc_hook()
    pn = nc.partition_id_tensor.name if nc.partition_id_tensor else None
    in_names, out_names, out_avals, out_shapes = [], [], [], []
    for alloc in nc.m.functions[0].allocations:
        if not isinstance(alloc, mb.MemoryLocationSet):
            continue
        name = alloc.memorylocations[0].name
        if alloc.kind == "ExternalInput":
            if name != pn:
                in_names.append(name)
        elif alloc.kind == "ExternalOutput":
            shape = tuple(alloc.tensor_shape)
            dtype = mb.dt.np(alloc.dtype)
            out_names.append(name)
            out_avals.append(jax.core.ShapedArray(shape, dtype))
            out_shapes.append((shape, dtype))
    n_params = len(in_names)
    n_outs = len(out_names)
    all_in = list(in_names) + list(out_names) + ([pn] if pn else [])

    def _body(*args):
        ops = list(args)
        if pn:
            ops.append(bass2jax.partition_id_tensor())
        return tuple(bass2jax._bass_exec_p.bind(
            *ops, out_avals=tuple(out_avals), in_names=tuple(all_in),
            out_names=tuple(out_names), lowering_input_output_aliases=(),
            sim_require_finite=True, sim_require_nnan=True, nc=nc))

    devices = jax.devices()[:n_cores]
    mesh = Mesh(np.asarray(devices), ("core",))
    specs = (PartitionSpec("core"),)
    fn = jax.jit(
        shard_map(_body, mesh=mesh, in_specs=specs * (n_params + n_outs),
                  out_specs=specs * n_outs, check_rep=False),
        donate_argnums=tuple(range(n_params, n_params + n_outs)) if donate else (),
        keep_unused=True)

    def run(in_maps):
        per_core = [[np.asarray(m[name]) for name in in_names] for m in in_maps]
        concat_in = [np.concatenate([per_core[c][i] for c in range(n_cores)],
                                    axis=0) for i in range(n_params)]
        concat_zeros = [np.zeros((n_cores * s[0], *s[1:]), d)
                        for (s, d) in out_shapes]
        import jax as _jax
        out_arrs = _jax.block_until_ready(fn(*concat_in, *concat_zeros))
        return [
            {name: np.asarray(out_arrs[i]).reshape(n_cores, *out_shapes[i][0])[c]
             for i, name in enumerate(out_names)}
            for c in range(n_cores)
        ]

    run.jit_fn = fn
    run.in_names = in_names
    run.out_names = out_names
    run.out_shapes = out_shapes
    run.n_cores = n_cores
    return run


def _unit_groups():
    units = [(b, r) for b in range(B) for r in range(A)]
    return [units[g * UNITS:(g + 1) * UNITS] for g in range(4)]


def shard_inputs(x, w_qkv, b_qkv, w_proj, b_proj):
    groups = _unit_groups()
    w4 = w_qkv.reshape(DIM, H, 3, D)
    b4 = b_qkv.reshape(H, 3, D)
    in_maps = []
    for c in range(NCORES):
        g, hh = c // 2, c % 2
        heads = list(range(hh * HL, (hh + 1) * HL))
        xT = np.ascontiguousarray(
            np.stack([x[b, :, r, :].T for (b, r) in groups[g]])
        ).astype(np.float32)
        wq = w4[:, heads, 0, :].reshape(DIM, HL * D) * SCALE
        wk = w4[:, heads, 1, :].reshape(DIM, HL * D) * SCALE
        wv = w4[:, heads, 2, :].reshape(DIM, HL * D)
        wqkv_c = np.ascontiguousarray(
            np.concatenate([wq, wk, wv], axis=1)).astype(np.float32)
        bq = (b4[heads, 0, :].reshape(HL * D) * SCALE)
        bk = (b4[heads, 1, :].reshape(HL * D) * SCALE)
        bvv = np.concatenate([b4[heads, 2, :], np.ones((HL, 1), np.float32)],
                             axis=1).reshape(HL * VW)
        in_maps.append({
            "xT": xT,
            "wqkv": wqkv_c,
            "bqk": np.concatenate([bq, bk]).astype(np.float32),
            "bv": bvv.astype(np.float32),
            "wproj": np.ascontiguousarray(
                w_proj[hh * HL * D:(hh + 1) * HL * D, :]).astype(np.float32),
        })
    return in_maps


def unshard(results, b_proj):
    groups = _unit_groups()
    out = np.zeros((B, N, A, DIM), np.float32)
    for g in range(4):
        s = results[2 * g]["part"] + results[2 * g + 1]["part"]
        for idx, (b, r) in enumerate(groups[g]):
            out[b, :, r, :] = s[idx]
    return out + b_proj.astype(np.float32)


def get_runner(qk_bias=False):
    key = ("runner", qk_bias)
    if key not in _CACHE:
        nc = _build_nc(qk_bias=qk_bias)
        _CACHE[key] = _make_runner(nc)
    return _CACHE[key]


def kernel(x, w_qkv, b_qkv, w_proj, b_proj):
    x = np.asarray(x)
    w_qkv = np.asarray(w_qkv)
    b_qkv = np.asarray(b_qkv)
    w_proj = np.asarray(w_proj)
    b_proj = np.asarray(b_proj)
    run = get_runner(qk_bias=bool(np.any(b_qkv[:2048])))
    in_maps = shard_inputs(x, w_qkv, b_qkv, w_proj, b_proj)
    results = run(in_maps)
    return unshard(results, b_proj)
